# revision 12
# baseline (speedup 1.0000x reference)
"""Trainium2 Bass kernel for nn_MechanicsFunctionsMultiBlock.

Computes per-element hessians of a Neo-Hookean energy (linear triangles,
one quadrature point) for 800k elements split into two material blocks.

Sharding (hardcoded per spec): elements are sharded across the 8
NeuronCores by material block — cores 0-3 take quarters of blocks0
(lam=1.0, mu=0.5), cores 4-7 quarters of blocks1 (lam=2.0, mu=1.0).
Per-element rows (shapeGrads / vols / state / conns-gathered U rows) are
gathered on the host while sharding; the output element-hessian array
stays sharded along the element axis so the final scatter is a plain
per-core block write.

Closed form used on device (validated to ~1e-7 rel against autodiff):
  G = shapeGrads[e,0]  (3x2),  u = U[conns[e]]  (3x2)
  gradU = u^T G,  F = I + gradU,  J = det F,  lnJ = ln J
  ghat = G adj(F)          (= J * G F^-1, no division)
  c1 = mu (1 + 0.01 q),  c2 = c1 - lam lnJ
  x = (vol / J^2) ghat,  S[n,m] = vol c1 (G G^T)[n,m]
  H[n,a,m,b] = S[n,m] d_ab + c2 x[n,b] ghat[m,a] + lam x[n,a] ghat[m,b]

Device schedule (one 128x784 SoA chunk per core, all fp32):
  All elementwise planes live in one 55-plane SBUF arena; the 21 unique
  hessian output planes are overlaid on the input planes (dead by the
  time H assembly starts). The 15 symmetric duplicate planes are never
  computed — the same SBUF planes are DMA'd twice to DRAM. Output DMAs
  are issued in waves as plane groups complete. Same-shape plane groups
  are fused into single wide strided/broadcast-AP vector ops (~68 DVE
  instructions for 113 plane traversals), and the twelve terminal "+S"
  accumulations run on the DMA engines' inline CCE adders (SBUF->SBUF
  accumulate DMAs) instead of the vector engine, which is the
  bottleneck at fp32 1x (~816 ns per 128x784 plane op, HW-calibrated).
"""
import numpy as np

import concourse.bass as bass
import concourse.tile as tile
from concourse import mybir
from concourse.bass_utils import run_bass_kernel_spmd
from concourse.vector_clock import ScopedClock, VectorClock

# ---------------------------------------------------------------- constants
E = 800_000
N = 400_000
MATS = ((1.0, 0.5), (2.0, 1.0))  # (lam, mu) for block0 / block1
NCORES = 8
K = E // 2 // 4            # 100_000 elements per core
PART = 128
FREE = 784                 # 128*784 = 100_352 padded elements per core
ELP = PART * FREE

F32 = mybir.dt.float32
ALU = mybir.AluOpType
ACTF = mybir.ActivationFunctionType

# ---- output plane order (also the arena slot order, po = slots 0..20) ----
# 6x6 hessian entry (r, c): r = 2n + a, c = 2m + b. Planes listed in the
# order they are computed; the last 15 hout slots repeat the first 15
# (their transposes), written from the same SBUF planes.
_NM = [(0, 1), (0, 2), (1, 2)]
PO_ORDER = (
    [rc for (n, m) in _NM for rc in ((2 * n, 2 * m + 1), (2 * n + 1, 2 * m))]
    + [(2 * n, 2 * n + 1) for n in range(3)]
    + [(2 * n + a, 2 * m + a) for (n, m) in _NM for a in range(2)]
    + [(r, r) for r in range(6)]
)
SLOT_RC = PO_ORDER + [(c, r) for (r, c) in PO_ORDER[:15]]  # 36 hout slots
_PLANE = {rc: i for i, rc in enumerate(PO_ORDER)}

# ---- arena slot map (55 planes of [128, 784] fp32) -----------------------
# 0..20  : po (21 output planes); 0..7 double as fin (g00..g21, vol, q),
#          8..13 double as u6 — inputs are fully consumed before the
#          first H plane is written.
# 21..24 : gu (gradU)   25..31 : misc   32..37 : ghat
# 38..43 : S            44..49 : x (→ x2 in place)   50..54 : temps
NSLOT = 55
_GU, _MISC, _GH, _SS, _X, _TMP = 21, 25, 32, 38, 44, 50


def _split_drain(tc_cls):
    """TileContext whose tail drain emits one sem wait per no-op.

    The walrus build here rejects instructions carrying more than one
    sync wait; TileContext's stock exit puts every live processor's
    final tick on a single Drain.
    """

    class SplitDrainTileContext(tc_cls):
        def _drain_and_barrier(self, tick_clock, wait_clock):
            ticks = list(tick_clock.global_clock)
            for i, t in enumerate(ticks):
                if t <= 0:
                    continue
                sub = [t if j == i else 0 for j in range(len(ticks))]
                nop = self.nc.sync.nop()
                wait_clock.add_sem_waits(nop.ins, ScopedClock({None: VectorClock(sub)}))
            self.nc.sync.drain()
            self.nc.all_engine_barrier()
            assert self.sems is not None
            popped = self.nc._tile_sem_poison_stack.pop()
            assert popped is self._sem_poison
            self.nc.clear_and_free_semaphores(list(self.sems.allocated().values()))
            self.nc.all_engine_barrier()

    return SplitDrainTileContext


def _legalize_single_wait(nc):
    """Split multi-wait instructions: this walrus build encodes at most one
    sync wait per instruction (two for EventSemaphore). Hoist extra waits
    onto same-engine no-ops inserted immediately before."""
    import bass_rust

    n = 0
    for fn in nc.m.functions:
        for blk in fn.blocks:
            out = []
            for ins in blk.instructions:
                si = ins.sync_info
                cap = 2 if isinstance(ins, mybir.InstEventSemaphore) else 1
                if si is not None and len(si.on_wait) > cap:
                    waits = list(si.on_wait)
                    for w in waits[:-cap]:
                        nop = mybir.InstNoOp(name=f"I-wsplit-{n}", ins=[], outs=[])
                        n += 1
                        nop.engine = ins.engine
                        nop.sync_info = bass_rust.SyncInfo(on_wait=[w], on_update=[])
                        out.append(nop)
                    ins.sync_info = bass_rust.SyncInfo(
                        on_wait=waits[-cap:], on_update=list(si.on_update)
                    )
                out.append(ins)
            blk.instructions = out


# ------------------------------------------------------------- bass program
def build_nc(legalize=True, variant="full"):
    lvl = {"dma": 0, "full": 3}[variant]
    nc = bass.Bass()
    fin = nc.declare_dram_parameter("fin", [8, PART, FREE], F32, isOutput=False)
    u6 = nc.declare_dram_parameter("u6", [6, PART, FREE], F32, isOutput=False)
    mats = nc.declare_dram_parameter("mats", [PART, 4], F32, isOutput=False)
    hout = nc.declare_dram_parameter("hout", [36, PART, FREE], F32, isOutput=True)

    TC = _split_drain(tile.TileContext)
    with TC(nc) as tc:
        with tc.tile_pool(name="arena_pool", bufs=1) as pool:
            mt = pool.tile([PART, 4], F32, name="mats_t", tag="mats_t")
            nc.sync.dma_start(out=mt[:], in_=mats[:])
            ap_lam, ap_mu = mt[:, 0:1], mt[:, 1:2]
            ap_mu001, ap_nlam = mt[:, 2:3], mt[:, 3:4]

            ar = pool.tile([PART, NSLOT, FREE], F32, name="arena", tag="arena")

            def sl(i):
                return ar[:, i, :]

            # -------- input DMAs: fin -> slots 0..7, u6 -> slots 8..13
            nc.sync.dma_start(
                out=ar[:, 0:8, :], in_=fin[:].rearrange("k p j -> p k j")
            )
            nc.sync.dma_start(
                out=ar[:, 8:14, :], in_=u6[:].rearrange("k p j -> p k j")
            )

            g = lambda n, i: sl(2 * n + i)
            vol, q = sl(6), sl(7)
            u = lambda n, cc: sl(8 + 2 * n + cc)
            h = lambda n, a, m, b: sl(_PLANE[(2 * n + a, 2 * m + b)])
            gu = lambda i: sl(_GU + i)
            ghat = lambda n, a: sl(_GH + 2 * n + a)
            PAIRS = [(0, 0), (1, 1), (2, 2), (0, 1), (0, 2), (1, 2)]
            S = lambda n, m: sl(_SS + PAIRS.index((min(n, m), max(n, m))))
            x = lambda n, a: sl(_X + 2 * n + a)
            F00, F11, J, lnJ, iJ2, c1, c2 = (sl(_MISC + i) for i in range(7))
            c2l, w2, vc1 = J, iJ2, c1  # slot reuse after J/iJ2/c1 die
            t0, P1, P2, q12a, q12b = (sl(_TMP + i) for i in range(5))

            TT = nc.vector.tensor_tensor
            ACT = nc.scalar.activation

            import dataclasses as _dc

            def pl(base, count, step=1):
                a = sl(base)
                return _dc.replace(a, ap=[a.ap[0], [step * FREE, count], [1, FREE]])

            def bc(plane, count):
                return _dc.replace(plane, ap=[plane.ap[0], [0, count], plane.ap[1]])

            if lvl >= 3:
                # ---- 1. S'[n,m] = (G G^T)[n,m] (needs fin only; overlaps
                #         the u6 input DMA). Diagonal trio fused.
                TT(out=pl(_SS, 3), in0=pl(0, 3, 2), in1=pl(0, 3, 2), op=ALU.mult)
                TT(out=pl(_X, 3), in0=pl(1, 3, 2), in1=pl(1, 3, 2), op=ALU.mult)
                TT(out=pl(_SS, 3), in0=pl(_SS, 3), in1=pl(_X, 3), op=ALU.add)
                for k, (n, m) in enumerate(_NM):
                    d = sl(_SS + 3 + k)
                    TT(out=d, in0=g(n, 0), in1=g(m, 0), op=ALU.mult)
                    TT(out=t0, in0=g(n, 1), in1=g(m, 1), op=ALU.mult)
                    TT(out=d, in0=d, in1=t0, op=ALU.add)

                # ---- 2. gradU via fused 3-plane products, temps in x region
                for cc in range(2):
                    for i in range(2):
                        TT(out=pl(_X + i * 3, 3),
                           in0=pl(8 + cc, 3, 2), in1=pl(i, 3, 2), op=ALU.mult)
                    TT(out=pl(_GU + 2 * cc, 2), in0=pl(_X, 2, 3),
                       in1=pl(_X + 1, 2, 3), op=ALU.add)
                    TT(out=pl(_GU + 2 * cc, 2), in0=pl(_GU + 2 * cc, 2),
                       in1=pl(_X + 2, 2, 3), op=ALU.add)
                gu00, gu01, gu10, gu11 = gu(0), gu(1), gu(2), gu(3)

                # ---- 3. F, J, transcendentals, coefficients
                ACT(out=F00, in_=gu00, func=ACTF.Identity, bias=1.0)
                ACT(out=F11, in_=gu11, func=ACTF.Identity, bias=1.0)
                TT(out=J, in0=F00, in1=F11, op=ALU.mult)
                TT(out=t0, in0=gu01, in1=gu10, op=ALU.mult)
                TT(out=J, in0=J, in1=t0, op=ALU.subtract)
                ACT(out=lnJ, in_=J, func=ACTF.Ln)
                ACT(out=iJ2, in_=lnJ, func=ACTF.Exp, scale=-2.0)
                ACT(out=c1, in_=q, func=ACTF.Identity, scale=ap_mu001, bias=ap_mu)
                ACT(out=c2, in_=lnJ, func=ACTF.Copy, scale=ap_nlam)
                TT(out=c2, in0=c2, in1=c1, op=ALU.add)
                # J dead (lnJ taken); c2l shares J's slot
                ACT(out=c2l, in_=c2, func=ACTF.Identity, bias=ap_lam)

                # ---- 4. ghat = G adj(F), fused per column
                TT(out=pl(_GH, 3, 2), in0=pl(0, 3, 2), in1=bc(F11, 3), op=ALU.mult)
                TT(out=pl(_X, 3), in0=pl(1, 3, 2), in1=bc(gu10, 3), op=ALU.mult)
                TT(out=pl(_GH, 3, 2), in0=pl(_GH, 3, 2), in1=pl(_X, 3), op=ALU.subtract)
                TT(out=pl(_GH + 1, 3, 2), in0=pl(1, 3, 2), in1=bc(F00, 3), op=ALU.mult)
                TT(out=pl(_X, 3), in0=pl(0, 3, 2), in1=bc(gu01, 3), op=ALU.mult)
                TT(out=pl(_GH + 1, 3, 2), in0=pl(_GH + 1, 3, 2), in1=pl(_X, 3), op=ALU.subtract)

                # ---- 5. w2 = vol/J^2, vc1 = vol*c1 (in place), S *= vc1, x = w2*ghat
                TT(out=w2, in0=vol, in1=iJ2, op=ALU.mult)
                TT(out=vc1, in0=vol, in1=c1, op=ALU.mult)
                TT(out=pl(_SS, 6), in0=pl(_SS, 6), in1=bc(vc1, 6), op=ALU.mult)
                TT(out=pl(_X, 6), in0=bc(w2, 6), in1=pl(_GH, 6), op=ALU.mult)

                # ---- 6. off-diagonal pairs (po slots 0..5)
                for n, m in _NM:
                    TT(out=P1, in0=x(n, 1), in1=ghat(m, 0), op=ALU.mult)
                    TT(out=P2, in0=x(n, 0), in1=ghat(m, 1), op=ALU.mult)
                    d = h(n, 0, m, 1)
                    TT(out=d, in0=P1, in1=c2, op=ALU.mult)
                    ACT(out=q12a, in_=P2, func=ACTF.Copy, scale=ap_lam)
                    TT(out=d, in0=d, in1=q12a, op=ALU.add)
                    d = h(n, 1, m, 0)
                    TT(out=d, in0=P2, in1=c2, op=ALU.mult)
                    ACT(out=q12b, in_=P1, func=ACTF.Copy, scale=ap_lam)
                    TT(out=d, in0=d, in1=q12b, op=ALU.add)

                # wave 1: off-diagonal planes + their transposes
                nc.sync.dma_start(
                    out=hout[0:6, :, :].rearrange("k p j -> p k j"),
                    in_=ar[:, 0:6, :],
                )
                nc.sync.dma_start(
                    out=hout[21:27, :, :].rearrange("k p j -> p k j"),
                    in_=ar[:, 0:6, :],
                )

                # ---- 7. x2 = c2l x (in place over x)
                TT(out=pl(_X, 6), in0=bc(c2l, 6), in1=pl(_X, 6), op=ALU.mult)
                x2 = x

                # H[n,0,n,1] = x2[n,0] ghat[n,1]   (po slots 6..8)
                TT(out=pl(6, 3), in0=pl(_X, 3, 2), in1=pl(_GH + 1, 3, 2), op=ALU.mult)
                # a == b, n < m: H = x2[n,a] ghat[m,a] + S[n,m]  (po 9..14)
                # products on DVE; the +S via DMA-engine CCE accumulate
                for k, (n, m) in enumerate(_NM):
                    for a in range(2):
                        d = h(n, a, m, a)
                        TT(out=d, in0=x2(n, a), in1=ghat(m, a), op=ALU.mult)
                for a in range(2):
                    nc.gpsimd.dma_start(
                        out=pl(9 + a, 3, 2), in_=pl(_SS + 3, 3), accum_op=ALU.add
                    )

                # wave 2: slots 6..14 + their transposes
                nc.sync.dma_start(
                    out=hout[6:15, :, :].rearrange("k p j -> p k j"),
                    in_=ar[:, 6:15, :],
                )
                nc.sync.dma_start(
                    out=hout[27:36, :, :].rearrange("k p j -> p k j"),
                    in_=ar[:, 6:15, :],
                )

                # ---- 8. diagonal: H[n,a,n,a] = x2[n,a] ghat[n,a] + S[n,n]
                # products on DVE; the +S via DMA-engine CCE accumulate
                TT(out=pl(15, 6), in0=pl(_X, 6), in1=pl(_GH, 6), op=ALU.mult)
                for a in range(2):
                    nc.gpsimd.dma_start(
                        out=pl(15 + a, 3, 2), in_=pl(_SS, 3), accum_op=ALU.add
                    )

                # wave 3: diagonal slots 15..20
                nc.sync.dma_start(
                    out=hout[15:21, :, :].rearrange("k p j -> p k j"),
                    in_=ar[:, 15:21, :],
                )
            else:
                nc.vector.memset(ar[:, 0:21, :], 0.0)
                nc.sync.dma_start(
                    out=hout[0:21, :, :].rearrange("k p j -> p k j"),
                    in_=ar[:, 0:21, :],
                )
                nc.sync.dma_start(
                    out=hout[21:36, :, :].rearrange("k p j -> p k j"),
                    in_=ar[:, 6:21, :],
                )
    if legalize:
        _legalize_single_wait(nc)
    return nc


_NC_CACHE = None


def _get_nc():
    global _NC_CACHE
    if _NC_CACHE is None:
        _NC_CACHE = build_nc()
    return _NC_CACHE


# ------------------------------------------------------------------- host
def _shard_core(U, state, conns, shapeGrads, vols, ids, lam, mu):
    KX = len(ids)
    g6 = shapeGrads[ids, 0].reshape(KX, 6)          # (n,i) C-order
    fin = np.zeros((8, ELP), np.float32)
    fin[:6, :KX] = g6.T
    fin[6, :KX] = vols[ids, 0]
    fin[7, :KX] = state[ids, 0, 0]

    uu = U[conns[ids]].reshape(KX, 6)               # (n,c) C-order
    u6 = np.zeros((6, ELP), np.float32)
    u6[:, :KX] = uu.T

    mats = np.empty((PART, 4), np.float32)
    mats[:, 0] = lam
    mats[:, 1] = mu
    mats[:, 2] = 0.01 * mu
    mats[:, 3] = -lam
    return {
        "fin": fin.reshape(8, PART, FREE),
        "u6": u6.reshape(6, PART, FREE),
        "mats": mats,
    }


_ROW_SEL = np.array([rc[0] for rc in SLOT_RC])
_COL_SEL = np.array([rc[1] for rc in SLOT_RC])


def _decode_core(hout):
    planes = np.asarray(hout).reshape(36, ELP)[:, :K]   # [36, K]
    Hm = np.empty((K, 6, 6), np.float32)
    Hm[:, _ROW_SEL, _COL_SEL] = planes.T
    return Hm.reshape(K, 3, 2, 3, 2)


def kernel(**inputs):
    U = np.asarray(inputs["U"], np.float32)
    state = np.asarray(inputs["state"], np.float32)
    conns = np.asarray(inputs["conns"])
    shapeGrads = np.asarray(inputs["shapeGrads"], np.float32)
    vols = np.asarray(inputs["vols"], np.float32)
    blocks = (np.asarray(inputs["blocks0"]), np.asarray(inputs["blocks1"]))

    core_ids = list(range(NCORES))
    in_maps = []
    id_lists = []
    for d in core_ids:
        blk, (lam, mu) = blocks[d // 4], MATS[d // 4]
        ids = blk[(d % 4) * K : (d % 4 + 1) * K]
        id_lists.append(ids)
        in_maps.append(_shard_core(U, state, conns, shapeGrads, vols, ids, lam, mu))

    res = run_bass_kernel_spmd(_get_nc(), in_maps, core_ids=core_ids)

    hess = np.empty((E, 3, 2, 3, 2), np.float32)
    for d in core_ids:
        hess[id_lists[d]] = _decode_core(res.results[d]["hout"])
    return hess


# revision 14
# speedup vs baseline: 1.1780x; 1.1780x over previous
"""Trainium2 Bass kernel for nn_MechanicsFunctionsMultiBlock.

Computes per-element hessians of a Neo-Hookean energy (linear triangles,
one quadrature point) for 800k elements split into two material blocks.

Sharding (hardcoded per spec): elements are sharded across the 8
NeuronCores by material block — cores 0-3 take quarters of blocks0
(lam=1.0, mu=0.5), cores 4-7 quarters of blocks1 (lam=2.0, mu=1.0).
Per-element rows (shapeGrads / vols / state / conns-gathered U rows) are
gathered on the host while sharding; the output element-hessian array
stays sharded along the element axis so the final scatter is a plain
per-core block write.

Closed form used on device (validated to ~1e-7 rel against autodiff):
  G = shapeGrads[e,0]  (3x2),  u = U[conns[e]]  (3x2)
  gradU = u^T G,  F = I + gradU,  J = det F,  lnJ = ln J
  ghat = G adj(F)          (= J * G F^-1, no division)
  c1 = mu (1 + 0.01 q),  c2 = c1 - lam lnJ
  x = (vol / J^2) ghat,  S[n,m] = vol c1 (G G^T)[n,m]
  H[n,a,m,b] = S[n,m] d_ab + c2 x[n,b] ghat[m,a] + lam x[n,a] ghat[m,b]

Device schedule (one 128x784 SoA chunk per core, all fp32):
  All elementwise planes live in one 55-plane SBUF arena; the 21 unique
  hessian output planes are overlaid on the input planes (dead by the
  time H assembly starts). The 15 symmetric duplicate planes are never
  computed — the same SBUF planes are DMA'd twice to DRAM. Output DMAs
  are issued in waves as plane groups complete. Same-shape plane groups
  are fused into single wide strided/broadcast-AP vector ops (~68 DVE
  instructions for 113 plane traversals), and the twelve terminal "+S"
  accumulations run on the DMA engines' inline CCE adders (SBUF->SBUF
  accumulate DMAs) instead of the vector engine, which is the
  bottleneck at fp32 1x (~816 ns per 128x784 plane op, HW-calibrated).
"""
import numpy as np

import concourse.bass as bass
import concourse.tile as tile
from concourse import mybir
from concourse.bass_utils import run_bass_kernel_spmd
from concourse.vector_clock import ScopedClock, VectorClock

# ---------------------------------------------------------------- constants
E = 800_000
N = 400_000
MATS = ((1.0, 0.5), (2.0, 1.0))  # (lam, mu) for block0 / block1
NCORES = 8
K = E // 2 // 4            # 100_000 elements per core
PART = 128
FREE = 784                 # 128*784 = 100_352 padded elements per core
ELP = PART * FREE

F32 = mybir.dt.float32
ALU = mybir.AluOpType
ACTF = mybir.ActivationFunctionType

# ---- output plane order (also the arena slot order, po = slots 0..20) ----
# 6x6 hessian entry (r, c): r = 2n + a, c = 2m + b. Planes listed in the
# order they are computed; the last 15 hout slots repeat the first 15
# (their transposes), written from the same SBUF planes.
_NM = [(0, 1), (0, 2), (1, 2)]
PO_ORDER = (
    [rc for (n, m) in _NM for rc in ((2 * n, 2 * m + 1), (2 * n + 1, 2 * m))]
    + [(2 * n, 2 * n + 1) for n in range(3)]
    + [(2 * n + a, 2 * m + a) for (n, m) in _NM for a in range(2)]
    + [(r, r) for r in range(6)]
)
SLOT_RC = PO_ORDER + [(c, r) for (r, c) in PO_ORDER[:15]]  # 36 hout slots
_PLANE = {rc: i for i, rc in enumerate(PO_ORDER)}

# ---- arena slot map (55 planes of [128, 784] fp32) -----------------------
# 0..20  : po (21 output planes); 0..7 double as fin (g00..g21, vol, q),
#          8..13 double as u6 — inputs are fully consumed before the
#          first H plane is written.
# 21..24 : gu (gradU)   25..31 : misc   32..37 : ghat
# 38..43 : S            44..49 : x (→ x2 in place)   50..54 : temps
NSLOT = 55
_GU, _MISC, _GH, _SS, _X, _TMP = 21, 25, 32, 38, 44, 50


def _split_drain(tc_cls):
    """TileContext whose tail drain emits one sem wait per no-op.

    The walrus build here rejects instructions carrying more than one
    sync wait; TileContext's stock exit puts every live processor's
    final tick on a single Drain.
    """

    class SplitDrainTileContext(tc_cls):
        def _drain_and_barrier(self, tick_clock, wait_clock):
            ticks = list(tick_clock.global_clock)
            for i, t in enumerate(ticks):
                if t <= 0:
                    continue
                sub = [t if j == i else 0 for j in range(len(ticks))]
                nop = self.nc.sync.nop()
                wait_clock.add_sem_waits(nop.ins, ScopedClock({None: VectorClock(sub)}))
            self.nc.sync.drain()
            self.nc.all_engine_barrier()
            assert self.sems is not None
            popped = self.nc._tile_sem_poison_stack.pop()
            assert popped is self._sem_poison
            self.nc.clear_and_free_semaphores(list(self.sems.allocated().values()))
            self.nc.all_engine_barrier()

    return SplitDrainTileContext


def _legalize_single_wait(nc):
    """Split multi-wait instructions: this walrus build encodes at most one
    sync wait per instruction (two for EventSemaphore). Hoist extra waits
    onto same-engine no-ops inserted immediately before."""
    import bass_rust

    n = 0
    for fn in nc.m.functions:
        for blk in fn.blocks:
            out = []
            for ins in blk.instructions:
                si = ins.sync_info
                cap = 2 if isinstance(ins, mybir.InstEventSemaphore) else 1
                if si is not None and len(si.on_wait) > cap:
                    waits = list(si.on_wait)
                    for w in waits[:-cap]:
                        nop = mybir.InstNoOp(name=f"I-wsplit-{n}", ins=[], outs=[])
                        n += 1
                        nop.engine = ins.engine
                        nop.sync_info = bass_rust.SyncInfo(on_wait=[w], on_update=[])
                        out.append(nop)
                    ins.sync_info = bass_rust.SyncInfo(
                        on_wait=waits[-cap:], on_update=list(si.on_update)
                    )
                out.append(ins)
            blk.instructions = out


# ------------------------------------------------------------- bass program
def build_nc(legalize=True, variant="full"):
    lvl = {"dma": 0, "full": 3}[variant]
    nc = bass.Bass()
    fin = nc.declare_dram_parameter("fin", [8, PART, FREE], F32, isOutput=False)
    u6 = nc.declare_dram_parameter("u6", [6, PART, FREE], F32, isOutput=False)
    mats = nc.declare_dram_parameter("mats", [PART, 4], F32, isOutput=False)
    hout = nc.declare_dram_parameter("hout", [36, PART, FREE], F32, isOutput=True)

    TC = _split_drain(tile.TileContext)
    with TC(nc) as tc:
        with tc.tile_pool(name="arena_pool", bufs=1) as pool:
            mt = pool.tile([PART, 4], F32, name="mats_t", tag="mats_t")
            ar = pool.tile([PART, NSLOT, FREE], F32, name="arena", tag="arena")

            def sl(i):
                return ar[:, i, :]

            # -------- input DMAs: fin -> slots 0..7, u6 -> slots 8..13
            # (big transfers first; the tiny mats load rides behind)
            nc.sync.dma_start(
                out=ar[:, 0:8, :], in_=fin[:].rearrange("k p j -> p k j")
            )
            nc.sync.dma_start(
                out=ar[:, 8:14, :], in_=u6[:].rearrange("k p j -> p k j")
            )
            nc.sync.dma_start(out=mt[:], in_=mats[:])
            ap_lam, ap_mu = mt[:, 0:1], mt[:, 1:2]
            ap_mu001, ap_nlam = mt[:, 2:3], mt[:, 3:4]

            g = lambda n, i: sl(2 * n + i)
            vol, q = sl(6), sl(7)
            u = lambda n, cc: sl(8 + 2 * n + cc)
            h = lambda n, a, m, b: sl(_PLANE[(2 * n + a, 2 * m + b)])
            gu = lambda i: sl(_GU + i)
            ghat = lambda n, a: sl(_GH + 2 * n + a)
            PAIRS = [(0, 0), (1, 1), (2, 2), (0, 1), (0, 2), (1, 2)]
            S = lambda n, m: sl(_SS + PAIRS.index((min(n, m), max(n, m))))
            x = lambda n, a: sl(_X + 2 * n + a)
            F00, F11, J, lnJ, iJ2, c1, c2 = (sl(_MISC + i) for i in range(7))
            c2l, w2, vc1 = J, iJ2, c1  # slot reuse after J/iJ2/c1 die
            t0, P1, P2, q12a, q12b = (sl(_TMP + i) for i in range(5))

            TT = nc.vector.tensor_tensor
            ACT = nc.scalar.activation

            import dataclasses as _dc

            def pl(base, count, step=1):
                a = sl(base)
                return _dc.replace(a, ap=[a.ap[0], [step * FREE, count], [1, FREE]])

            def bc(plane, count):
                return _dc.replace(plane, ap=[plane.ap[0], [0, count], plane.ap[1]])

            if lvl >= 3:
                # ---- 1. S'[n,m] = (G G^T)[n,m] (needs fin only; overlaps
                #         the u6 input DMA). Diagonal trio fused.
                TT(out=pl(_SS, 3), in0=pl(0, 3, 2), in1=pl(0, 3, 2), op=ALU.mult)
                TT(out=pl(_X, 3), in0=pl(1, 3, 2), in1=pl(1, 3, 2), op=ALU.mult)
                TT(out=pl(_SS, 3), in0=pl(_SS, 3), in1=pl(_X, 3), op=ALU.add)
                for k, (n, m) in enumerate(_NM):
                    d = sl(_SS + 3 + k)
                    TT(out=d, in0=g(n, 0), in1=g(m, 0), op=ALU.mult)
                    TT(out=t0, in0=g(n, 1), in1=g(m, 1), op=ALU.mult)
                    TT(out=d, in0=d, in1=t0, op=ALU.add)

                # ---- 2. gradU via fused 3-plane products, temps in x region
                for cc in range(2):
                    for i in range(2):
                        TT(out=pl(_X + i * 3, 3),
                           in0=pl(8 + cc, 3, 2), in1=pl(i, 3, 2), op=ALU.mult)
                    TT(out=pl(_GU + 2 * cc, 2), in0=pl(_X, 2, 3),
                       in1=pl(_X + 1, 2, 3), op=ALU.add)
                    TT(out=pl(_GU + 2 * cc, 2), in0=pl(_GU + 2 * cc, 2),
                       in1=pl(_X + 2, 2, 3), op=ALU.add)
                gu00, gu01, gu10, gu11 = gu(0), gu(1), gu(2), gu(3)

                # ---- 3. F, J, transcendentals, coefficients
                ACT(out=F00, in_=gu00, func=ACTF.Identity, bias=1.0)
                ACT(out=F11, in_=gu11, func=ACTF.Identity, bias=1.0)
                TT(out=J, in0=F00, in1=F11, op=ALU.mult)
                TT(out=t0, in0=gu01, in1=gu10, op=ALU.mult)
                TT(out=J, in0=J, in1=t0, op=ALU.subtract)
                ACT(out=lnJ, in_=J, func=ACTF.Ln)
                ACT(out=iJ2, in_=lnJ, func=ACTF.Exp, scale=-2.0)
                ACT(out=c1, in_=q, func=ACTF.Identity, scale=ap_mu001, bias=ap_mu)
                ACT(out=c2, in_=lnJ, func=ACTF.Copy, scale=ap_nlam)
                TT(out=c2, in0=c2, in1=c1, op=ALU.add)
                # J dead (lnJ taken); c2l shares J's slot
                ACT(out=c2l, in_=c2, func=ACTF.Identity, bias=ap_lam)

                # ---- 4. ghat = G adj(F), fused per column
                TT(out=pl(_GH, 3, 2), in0=pl(0, 3, 2), in1=bc(F11, 3), op=ALU.mult)
                TT(out=pl(_X, 3), in0=pl(1, 3, 2), in1=bc(gu10, 3), op=ALU.mult)
                TT(out=pl(_GH, 3, 2), in0=pl(_GH, 3, 2), in1=pl(_X, 3), op=ALU.subtract)
                TT(out=pl(_GH + 1, 3, 2), in0=pl(1, 3, 2), in1=bc(F00, 3), op=ALU.mult)
                TT(out=pl(_X, 3), in0=pl(0, 3, 2), in1=bc(gu01, 3), op=ALU.mult)
                TT(out=pl(_GH + 1, 3, 2), in0=pl(_GH + 1, 3, 2), in1=pl(_X, 3), op=ALU.subtract)

                # ---- 5. w2 = vol/J^2, vc1 = vol*c1 (in place), S *= vc1, x = w2*ghat
                TT(out=w2, in0=vol, in1=iJ2, op=ALU.mult)
                TT(out=vc1, in0=vol, in1=c1, op=ALU.mult)
                TT(out=pl(_SS, 6), in0=pl(_SS, 6), in1=bc(vc1, 6), op=ALU.mult)
                TT(out=pl(_X, 6), in0=bc(w2, 6), in1=pl(_GH, 6), op=ALU.mult)

                # ---- 6. off-diagonal pairs (po slots 0..5)
                for n, m in _NM:
                    TT(out=P1, in0=x(n, 1), in1=ghat(m, 0), op=ALU.mult)
                    TT(out=P2, in0=x(n, 0), in1=ghat(m, 1), op=ALU.mult)
                    d = h(n, 0, m, 1)
                    TT(out=d, in0=P1, in1=c2, op=ALU.mult)
                    ACT(out=q12a, in_=P2, func=ACTF.Copy, scale=ap_lam)
                    TT(out=d, in0=d, in1=q12a, op=ALU.add)
                    d = h(n, 1, m, 0)
                    TT(out=d, in0=P2, in1=c2, op=ALU.mult)
                    ACT(out=q12b, in_=P1, func=ACTF.Copy, scale=ap_lam)
                    TT(out=d, in0=d, in1=q12b, op=ALU.add)

                # wave 1: off-diagonal planes + their transposes
                nc.sync.dma_start(
                    out=hout[0:6, :, :].rearrange("k p j -> p k j"),
                    in_=ar[:, 0:6, :],
                )
                nc.sync.dma_start(
                    out=hout[21:27, :, :].rearrange("k p j -> p k j"),
                    in_=ar[:, 0:6, :],
                )

                # ---- 7. x2 = c2l x (in place over x)
                TT(out=pl(_X, 6), in0=bc(c2l, 6), in1=pl(_X, 6), op=ALU.mult)
                x2 = x

                # H[n,0,n,1] = x2[n,0] ghat[n,1]   (po slots 6..8)
                TT(out=pl(6, 3), in0=pl(_X, 3, 2), in1=pl(_GH + 1, 3, 2), op=ALU.mult)
                # wave 2a: slots 6..8 + their transposes ship immediately
                nc.sync.dma_start(
                    out=hout[6:9, :, :].rearrange("k p j -> p k j"),
                    in_=ar[:, 6:9, :],
                )
                nc.sync.dma_start(
                    out=hout[27:30, :, :].rearrange("k p j -> p k j"),
                    in_=ar[:, 6:9, :],
                )
                # a == b, n < m: H = x2[n,a] ghat[m,a] + S[n,m]  (po 9..14)
                for k, (n, m) in enumerate(_NM):
                    for a in range(2):
                        d = h(n, a, m, a)
                        TT(out=d, in0=x2(n, a), in1=ghat(m, a), op=ALU.mult)
                        TT(out=d, in0=d, in1=sl(_SS + 3 + k), op=ALU.add)

                # wave 2b: slots 9..14 + their transposes
                nc.sync.dma_start(
                    out=hout[9:15, :, :].rearrange("k p j -> p k j"),
                    in_=ar[:, 9:15, :],
                )
                nc.sync.dma_start(
                    out=hout[30:36, :, :].rearrange("k p j -> p k j"),
                    in_=ar[:, 9:15, :],
                )

                # ---- 8. diagonal: H[n,a,n,a] = x2[n,a] ghat[n,a] + S[n,n]
                TT(out=pl(15, 6), in0=pl(_X, 6), in1=pl(_GH, 6), op=ALU.mult)
                p15 = sl(15)
                d4 = _dc.replace(p15, ap=[p15.ap[0], [2 * FREE, 3], [FREE, 2], [1, FREE]])
                s38 = sl(_SS)
                s4 = _dc.replace(s38, ap=[s38.ap[0], [FREE, 3], [0, 2], [1, FREE]])
                TT(out=d4, in0=d4, in1=s4, op=ALU.add)

                # wave 3: diagonal slots 15..20
                nc.sync.dma_start(
                    out=hout[15:21, :, :].rearrange("k p j -> p k j"),
                    in_=ar[:, 15:21, :],
                )
            else:
                nc.vector.memset(ar[:, 0:21, :], 0.0)
                nc.sync.dma_start(
                    out=hout[0:21, :, :].rearrange("k p j -> p k j"),
                    in_=ar[:, 0:21, :],
                )
                nc.sync.dma_start(
                    out=hout[21:36, :, :].rearrange("k p j -> p k j"),
                    in_=ar[:, 6:21, :],
                )
    if legalize:
        _legalize_single_wait(nc)
    return nc


_NC_CACHE = None


def _get_nc():
    global _NC_CACHE
    if _NC_CACHE is None:
        _NC_CACHE = build_nc()
    return _NC_CACHE


# ------------------------------------------------------------------- host
def _shard_core(U, state, conns, shapeGrads, vols, ids, lam, mu):
    KX = len(ids)
    g6 = shapeGrads[ids, 0].reshape(KX, 6)          # (n,i) C-order
    fin = np.zeros((8, ELP), np.float32)
    fin[:6, :KX] = g6.T
    fin[6, :KX] = vols[ids, 0]
    fin[7, :KX] = state[ids, 0, 0]

    uu = U[conns[ids]].reshape(KX, 6)               # (n,c) C-order
    u6 = np.zeros((6, ELP), np.float32)
    u6[:, :KX] = uu.T

    mats = np.empty((PART, 4), np.float32)
    mats[:, 0] = lam
    mats[:, 1] = mu
    mats[:, 2] = 0.01 * mu
    mats[:, 3] = -lam
    return {
        "fin": fin.reshape(8, PART, FREE),
        "u6": u6.reshape(6, PART, FREE),
        "mats": mats,
    }


_ROW_SEL = np.array([rc[0] for rc in SLOT_RC])
_COL_SEL = np.array([rc[1] for rc in SLOT_RC])


def _decode_core(hout):
    planes = np.asarray(hout).reshape(36, ELP)[:, :K]   # [36, K]
    Hm = np.empty((K, 6, 6), np.float32)
    Hm[:, _ROW_SEL, _COL_SEL] = planes.T
    return Hm.reshape(K, 3, 2, 3, 2)


def kernel(**inputs):
    U = np.asarray(inputs["U"], np.float32)
    state = np.asarray(inputs["state"], np.float32)
    conns = np.asarray(inputs["conns"])
    shapeGrads = np.asarray(inputs["shapeGrads"], np.float32)
    vols = np.asarray(inputs["vols"], np.float32)
    blocks = (np.asarray(inputs["blocks0"]), np.asarray(inputs["blocks1"]))

    core_ids = list(range(NCORES))
    in_maps = []
    id_lists = []
    for d in core_ids:
        blk, (lam, mu) = blocks[d // 4], MATS[d // 4]
        ids = blk[(d % 4) * K : (d % 4 + 1) * K]
        id_lists.append(ids)
        in_maps.append(_shard_core(U, state, conns, shapeGrads, vols, ids, lam, mu))

    res = run_bass_kernel_spmd(_get_nc(), in_maps, core_ids=core_ids)

    hess = np.empty((E, 3, 2, 3, 2), np.float32)
    for d in core_ids:
        hess[id_lists[d]] = _decode_core(res.results[d]["hout"])
    return hess


# revision 17
# speedup vs baseline: 1.4276x; 1.2119x over previous
"""Trainium2 Bass kernel for nn_MechanicsFunctionsMultiBlock.

Computes per-element hessians of a Neo-Hookean energy (linear triangles,
one quadrature point) for 800k elements split into two material blocks.

Sharding (hardcoded per spec): elements are sharded across the 8
NeuronCores by material block — cores 0-3 take quarters of blocks0
(lam=1.0, mu=0.5), cores 4-7 quarters of blocks1 (lam=2.0, mu=1.0).
Per-element rows (shapeGrads / vols / state / conns-gathered U rows) are
gathered on the host while sharding; the output element-hessian array
stays sharded along the element axis so the final scatter is a plain
per-core block write.

Closed form used on device (validated to ~1e-7 rel against autodiff):
  G = shapeGrads[e,0]  (3x2),  u = U[conns[e]]  (3x2)
  gradU = u^T G,  F = I + gradU,  J = det F,  lnJ = ln J
  ghat = G adj(F)          (= J * G F^-1, no division)
  c1 = mu (1 + 0.01 q),  c2 = c1 - lam lnJ
  x = (vol / J^2) ghat,  S[n,m] = vol c1 (G G^T)[n,m]
  H[n,a,m,b] = S[n,m] d_ab + c2 x[n,b] ghat[m,a] + lam x[n,a] ghat[m,b]

Device schedule (one 128x784 SoA chunk per core, all fp32):
  All elementwise planes live in one 55-plane SBUF arena; the 21 unique
  hessian output planes are overlaid on the input planes (dead by the
  time H assembly starts). The 15 symmetric duplicate planes are never
  computed — the same SBUF planes are DMA'd twice to DRAM. Output DMAs
  are issued in five waves as plane groups complete, overlapping the
  vector engine, which is the bottleneck at fp32 1x (~816 ns per
  128x784 plane traversal, HW-calibrated). Same-shape plane groups are
  fused into single wide strided/broadcast-AP vector ops (~62 DVE
  instructions for 125 plane traversals). DMA-engine CCE accumulation
  was tried for the terminal "+S" adds and reverted: its 3-streams-per-
  plane SBUF traffic lands in the output tail and costs more than the
  vector time it saves.
"""
import numpy as np

import concourse.bass as bass
import concourse.tile as tile
from concourse import mybir
from concourse.bass_utils import run_bass_kernel_spmd
from concourse.vector_clock import ScopedClock, VectorClock

# ---------------------------------------------------------------- constants
E = 800_000
N = 400_000
MATS = ((1.0, 0.5), (2.0, 1.0))  # (lam, mu) for block0 / block1
NCORES = 8
K = E // 2 // 4            # 100_000 elements per core
PART = 128
FREE = 784                 # 128*784 = 100_352 padded elements per core
ELP = PART * FREE

F32 = mybir.dt.float32
ALU = mybir.AluOpType
ACTF = mybir.ActivationFunctionType

# ---- output plane order (also the arena slot order, po = slots 0..20) ----
# 6x6 hessian entry (r, c): r = 2n + a, c = 2m + b. Planes listed in the
# order they are computed; the last 15 hout slots repeat the first 15
# (their transposes), written from the same SBUF planes.
_NM = [(0, 1), (0, 2), (1, 2)]
PO_ORDER = (
    [rc for (n, m) in _NM for rc in ((2 * n, 2 * m + 1), (2 * n + 1, 2 * m))]
    + [(2 * n, 2 * n + 1) for n in range(3)]
    + [(2 * n + a, 2 * m + a) for (n, m) in _NM for a in range(2)]
    + [(r, r) for r in range(6)]
)
SLOT_RC = PO_ORDER + [(c, r) for (r, c) in PO_ORDER[:15]]  # 36 hout slots
_PLANE = {rc: i for i, rc in enumerate(PO_ORDER)}

# ---- arena slot map (55 planes of [128, 784] fp32) -----------------------
# 0..20  : po (21 output planes); 0..7 double as fin (g00..g21, vol, q),
#          8..13 double as u6 — inputs are fully consumed before the
#          first H plane is written.
# 21..24 : gu (gradU)   25..31 : misc   32..37 : ghat
# 38..43 : S            44..49 : x (→ x2 in place)   50..54 : temps
NSLOT = 55
_GU, _MISC, _GH, _SS, _X, _TMP = 21, 25, 32, 38, 44, 50


def _split_drain(tc_cls):
    """TileContext whose tail drain emits one sem wait per no-op.

    The walrus build here rejects instructions carrying more than one
    sync wait; TileContext's stock exit puts every live processor's
    final tick on a single Drain.
    """

    class SplitDrainTileContext(tc_cls):
        def _drain_and_barrier(self, tick_clock, wait_clock):
            ticks = list(tick_clock.global_clock)
            for i, t in enumerate(ticks):
                if t <= 0:
                    continue
                sub = [t if j == i else 0 for j in range(len(ticks))]
                nop = self.nc.sync.nop()
                wait_clock.add_sem_waits(nop.ins, ScopedClock({None: VectorClock(sub)}))
            self.nc.sync.drain()
            self.nc.all_engine_barrier()
            assert self.sems is not None
            popped = self.nc._tile_sem_poison_stack.pop()
            assert popped is self._sem_poison
            self.nc.clear_and_free_semaphores(list(self.sems.allocated().values()))
            self.nc.all_engine_barrier()

    return SplitDrainTileContext


def _legalize_single_wait(nc):
    """Split multi-wait instructions: this walrus build encodes at most one
    sync wait per instruction (two for EventSemaphore). Hoist extra waits
    onto same-engine no-ops inserted immediately before."""
    import bass_rust

    n = 0
    for fn in nc.m.functions:
        for blk in fn.blocks:
            out = []
            for ins in blk.instructions:
                si = ins.sync_info
                cap = 2 if isinstance(ins, mybir.InstEventSemaphore) else 1
                if si is not None and len(si.on_wait) > cap:
                    waits = list(si.on_wait)
                    for w in waits[:-cap]:
                        nop = mybir.InstNoOp(name=f"I-wsplit-{n}", ins=[], outs=[])
                        n += 1
                        nop.engine = ins.engine
                        nop.sync_info = bass_rust.SyncInfo(on_wait=[w], on_update=[])
                        out.append(nop)
                    ins.sync_info = bass_rust.SyncInfo(
                        on_wait=waits[-cap:], on_update=list(si.on_update)
                    )
                out.append(ins)
            blk.instructions = out


# ------------------------------------------------------------- bass program
def build_nc(legalize=True, variant="full"):
    lvl = {"dma": 0, "full": 3}[variant]
    nc = bass.Bass()
    fin = nc.declare_dram_parameter("fin", [8, PART, FREE], F32, isOutput=False)
    u6 = nc.declare_dram_parameter("u6", [6, PART, FREE], F32, isOutput=False)
    mats = nc.declare_dram_parameter("mats", [PART, 4], F32, isOutput=False)
    hout = nc.declare_dram_parameter("hout", [36, PART, FREE], F32, isOutput=True)

    TC = _split_drain(tile.TileContext)
    with TC(nc) as tc:
        with tc.tile_pool(name="arena_pool", bufs=1) as pool:
            mt = pool.tile([PART, 4], F32, name="mats_t", tag="mats_t")
            ar = pool.tile([PART, NSLOT, FREE], F32, name="arena", tag="arena")

            def sl(i):
                return ar[:, i, :]

            # -------- input DMAs: fin -> slots 0..7, u6 -> slots 8..13
            # (big transfers first; the tiny mats load rides behind)
            nc.sync.dma_start(
                out=ar[:, 0:8, :], in_=fin[:].rearrange("k p j -> p k j")
            )
            nc.sync.dma_start(
                out=ar[:, 8:14, :], in_=u6[:].rearrange("k p j -> p k j")
            )
            nc.sync.dma_start(out=mt[:], in_=mats[:])
            ap_lam, ap_mu = mt[:, 0:1], mt[:, 1:2]
            ap_mu001, ap_nlam = mt[:, 2:3], mt[:, 3:4]

            g = lambda n, i: sl(2 * n + i)
            vol, q = sl(6), sl(7)
            u = lambda n, cc: sl(8 + 2 * n + cc)
            h = lambda n, a, m, b: sl(_PLANE[(2 * n + a, 2 * m + b)])
            gu = lambda i: sl(_GU + i)
            ghat = lambda n, a: sl(_GH + 2 * n + a)
            PAIRS = [(0, 0), (1, 1), (2, 2), (0, 1), (0, 2), (1, 2)]
            S = lambda n, m: sl(_SS + PAIRS.index((min(n, m), max(n, m))))
            x = lambda n, a: sl(_X + 2 * n + a)
            F00, F11, J, lnJ, iJ2, c1, c2 = (sl(_MISC + i) for i in range(7))
            c2l, w2, vc1 = J, iJ2, c1  # slot reuse after J/iJ2/c1 die
            t0, P1, P2, q12a, q12b = (sl(_TMP + i) for i in range(5))

            TT = nc.vector.tensor_tensor
            ACT = nc.scalar.activation

            import dataclasses as _dc

            def pl(base, count, step=1):
                a = sl(base)
                return _dc.replace(a, ap=[a.ap[0], [step * FREE, count], [1, FREE]])

            def bc(plane, count):
                return _dc.replace(plane, ap=[plane.ap[0], [0, count], plane.ap[1]])

            if lvl >= 3:
                # ---- 1. S'[n,m] = (G G^T)[n,m] (needs fin only; overlaps
                #         the u6 input DMA). Diagonal trio fused.
                TT(out=pl(_SS, 3), in0=pl(0, 3, 2), in1=pl(0, 3, 2), op=ALU.mult)
                TT(out=pl(_X, 3), in0=pl(1, 3, 2), in1=pl(1, 3, 2), op=ALU.mult)
                TT(out=pl(_SS, 3), in0=pl(_SS, 3), in1=pl(_X, 3), op=ALU.add)
                for k, (n, m) in enumerate(_NM):
                    d = sl(_SS + 3 + k)
                    TT(out=d, in0=g(n, 0), in1=g(m, 0), op=ALU.mult)
                    TT(out=t0, in0=g(n, 1), in1=g(m, 1), op=ALU.mult)
                    TT(out=d, in0=d, in1=t0, op=ALU.add)

                # ---- 2. gradU via fused 3-plane products, temps in x region
                for cc in range(2):
                    for i in range(2):
                        TT(out=pl(_X + i * 3, 3),
                           in0=pl(8 + cc, 3, 2), in1=pl(i, 3, 2), op=ALU.mult)
                    TT(out=pl(_GU + 2 * cc, 2), in0=pl(_X, 2, 3),
                       in1=pl(_X + 1, 2, 3), op=ALU.add)
                    TT(out=pl(_GU + 2 * cc, 2), in0=pl(_GU + 2 * cc, 2),
                       in1=pl(_X + 2, 2, 3), op=ALU.add)
                gu00, gu01, gu10, gu11 = gu(0), gu(1), gu(2), gu(3)

                # ---- 3. F, J, transcendentals, coefficients
                ACT(out=F00, in_=gu00, func=ACTF.Identity, bias=1.0)
                ACT(out=F11, in_=gu11, func=ACTF.Identity, bias=1.0)
                TT(out=J, in0=F00, in1=F11, op=ALU.mult)
                TT(out=t0, in0=gu01, in1=gu10, op=ALU.mult)
                TT(out=J, in0=J, in1=t0, op=ALU.subtract)
                ACT(out=lnJ, in_=J, func=ACTF.Ln)
                ACT(out=iJ2, in_=lnJ, func=ACTF.Exp, scale=-2.0)
                ACT(out=c1, in_=q, func=ACTF.Identity, scale=ap_mu001, bias=ap_mu)
                ACT(out=c2, in_=lnJ, func=ACTF.Copy, scale=ap_nlam)
                TT(out=c2, in0=c2, in1=c1, op=ALU.add)
                # J dead (lnJ taken); c2l shares J's slot
                ACT(out=c2l, in_=c2, func=ACTF.Identity, bias=ap_lam)

                # ---- 4. ghat = G adj(F), fused per column
                TT(out=pl(_GH, 3, 2), in0=pl(0, 3, 2), in1=bc(F11, 3), op=ALU.mult)
                TT(out=pl(_X, 3), in0=pl(1, 3, 2), in1=bc(gu10, 3), op=ALU.mult)
                TT(out=pl(_GH, 3, 2), in0=pl(_GH, 3, 2), in1=pl(_X, 3), op=ALU.subtract)
                TT(out=pl(_GH + 1, 3, 2), in0=pl(1, 3, 2), in1=bc(F00, 3), op=ALU.mult)
                TT(out=pl(_X, 3), in0=pl(0, 3, 2), in1=bc(gu01, 3), op=ALU.mult)
                TT(out=pl(_GH + 1, 3, 2), in0=pl(_GH + 1, 3, 2), in1=pl(_X, 3), op=ALU.subtract)

                # ---- 5. w2 = vol/J^2, vc1 = vol*c1 (in place), S *= vc1, x = w2*ghat
                TT(out=w2, in0=vol, in1=iJ2, op=ALU.mult)
                TT(out=vc1, in0=vol, in1=c1, op=ALU.mult)
                TT(out=pl(_SS, 6), in0=pl(_SS, 6), in1=bc(vc1, 6), op=ALU.mult)
                TT(out=pl(_X, 6), in0=bc(w2, 6), in1=pl(_GH, 6), op=ALU.mult)

                # ---- 6. off-diagonal pairs (po slots 0..5)
                for n, m in _NM:
                    TT(out=P1, in0=x(n, 1), in1=ghat(m, 0), op=ALU.mult)
                    TT(out=P2, in0=x(n, 0), in1=ghat(m, 1), op=ALU.mult)
                    d = h(n, 0, m, 1)
                    TT(out=d, in0=P1, in1=c2, op=ALU.mult)
                    ACT(out=q12a, in_=P2, func=ACTF.Copy, scale=ap_lam)
                    TT(out=d, in0=d, in1=q12a, op=ALU.add)
                    d = h(n, 1, m, 0)
                    TT(out=d, in0=P2, in1=c2, op=ALU.mult)
                    ACT(out=q12b, in_=P1, func=ACTF.Copy, scale=ap_lam)
                    TT(out=d, in0=d, in1=q12b, op=ALU.add)

                # wave 1: off-diagonal planes + their transposes
                nc.sync.dma_start(
                    out=hout[0:6, :, :].rearrange("k p j -> p k j"),
                    in_=ar[:, 0:6, :],
                )
                nc.sync.dma_start(
                    out=hout[21:27, :, :].rearrange("k p j -> p k j"),
                    in_=ar[:, 0:6, :],
                )

                # ---- 7. x2 = c2l x (in place over x)
                TT(out=pl(_X, 6), in0=bc(c2l, 6), in1=pl(_X, 6), op=ALU.mult)
                x2 = x

                # H[n,0,n,1] = x2[n,0] ghat[n,1]   (po slots 6..8)
                TT(out=pl(6, 3), in0=pl(_X, 3, 2), in1=pl(_GH + 1, 3, 2), op=ALU.mult)
                # wave 2a: slots 6..8 + their transposes ship immediately
                nc.sync.dma_start(
                    out=hout[6:9, :, :].rearrange("k p j -> p k j"),
                    in_=ar[:, 6:9, :],
                )
                nc.sync.dma_start(
                    out=hout[27:30, :, :].rearrange("k p j -> p k j"),
                    in_=ar[:, 6:9, :],
                )
                # a == b, n < m: H = x2[n,a] ghat[m,a] + S[n,m]  (po 9..14)
                for k, (n, m) in enumerate(_NM):
                    for a in range(2):
                        d = h(n, a, m, a)
                        TT(out=d, in0=x2(n, a), in1=ghat(m, a), op=ALU.mult)
                        TT(out=d, in0=d, in1=sl(_SS + 3 + k), op=ALU.add)

                # wave 2b: slots 9..14 + their transposes
                nc.sync.dma_start(
                    out=hout[9:15, :, :].rearrange("k p j -> p k j"),
                    in_=ar[:, 9:15, :],
                )
                nc.sync.dma_start(
                    out=hout[30:36, :, :].rearrange("k p j -> p k j"),
                    in_=ar[:, 9:15, :],
                )

                # ---- 8. diagonal: H[n,a,n,a] = x2[n,a] ghat[n,a] + S[n,n]
                TT(out=pl(15, 6), in0=pl(_X, 6), in1=pl(_GH, 6), op=ALU.mult)
                p15 = sl(15)
                d4 = _dc.replace(p15, ap=[p15.ap[0], [2 * FREE, 3], [FREE, 2], [1, FREE]])
                s38 = sl(_SS)
                s4 = _dc.replace(s38, ap=[s38.ap[0], [FREE, 3], [0, 2], [1, FREE]])
                TT(out=d4, in0=d4, in1=s4, op=ALU.add)

                # wave 3: diagonal slots 15..20
                nc.sync.dma_start(
                    out=hout[15:21, :, :].rearrange("k p j -> p k j"),
                    in_=ar[:, 15:21, :],
                )
            else:
                nc.vector.memset(ar[:, 0:21, :], 0.0)
                nc.sync.dma_start(
                    out=hout[0:21, :, :].rearrange("k p j -> p k j"),
                    in_=ar[:, 0:21, :],
                )
                nc.sync.dma_start(
                    out=hout[21:36, :, :].rearrange("k p j -> p k j"),
                    in_=ar[:, 6:21, :],
                )
    if legalize:
        _legalize_single_wait(nc)
    return nc


_NC_CACHE = None


def _get_nc():
    global _NC_CACHE
    if _NC_CACHE is None:
        _NC_CACHE = build_nc()
    return _NC_CACHE


# ------------------------------------------------------------------- host
def _shard_core(U, state, conns, shapeGrads, vols, ids, lam, mu):
    KX = len(ids)
    g6 = shapeGrads[ids, 0].reshape(KX, 6)          # (n,i) C-order
    fin = np.zeros((8, ELP), np.float32)
    fin[:6, :KX] = g6.T
    fin[6, :KX] = vols[ids, 0]
    fin[7, :KX] = state[ids, 0, 0]

    uu = U[conns[ids]].reshape(KX, 6)               # (n,c) C-order
    u6 = np.zeros((6, ELP), np.float32)
    u6[:, :KX] = uu.T

    mats = np.empty((PART, 4), np.float32)
    mats[:, 0] = lam
    mats[:, 1] = mu
    mats[:, 2] = 0.01 * mu
    mats[:, 3] = -lam
    return {
        "fin": fin.reshape(8, PART, FREE),
        "u6": u6.reshape(6, PART, FREE),
        "mats": mats,
    }


_ROW_SEL = np.array([rc[0] for rc in SLOT_RC])
_COL_SEL = np.array([rc[1] for rc in SLOT_RC])


def _decode_core(hout):
    planes = np.asarray(hout).reshape(36, ELP)[:, :K]   # [36, K]
    Hm = np.empty((K, 6, 6), np.float32)
    Hm[:, _ROW_SEL, _COL_SEL] = planes.T
    return Hm.reshape(K, 3, 2, 3, 2)


def kernel(**inputs):
    U = np.asarray(inputs["U"], np.float32)
    state = np.asarray(inputs["state"], np.float32)
    conns = np.asarray(inputs["conns"])
    shapeGrads = np.asarray(inputs["shapeGrads"], np.float32)
    vols = np.asarray(inputs["vols"], np.float32)
    blocks = (np.asarray(inputs["blocks0"]), np.asarray(inputs["blocks1"]))

    core_ids = list(range(NCORES))
    in_maps = []
    id_lists = []
    for d in core_ids:
        blk, (lam, mu) = blocks[d // 4], MATS[d // 4]
        ids = blk[(d % 4) * K : (d % 4 + 1) * K]
        id_lists.append(ids)
        in_maps.append(_shard_core(U, state, conns, shapeGrads, vols, ids, lam, mu))

    res = run_bass_kernel_spmd(_get_nc(), in_maps, core_ids=core_ids)

    hess = np.empty((E, 3, 2, 3, 2), np.float32)
    for d in core_ids:
        hess[id_lists[d]] = _decode_core(res.results[d]["hout"])
    return hess
